# revision 52
# baseline (speedup 1.0000x reference)
"""Distributed Bass kernel for nn_AllLoss: YOLACT-style loss over 8 cores.

Per-core (one image each):
  cls:  -ln(p[pos]).sum()/K/K  +  -ln(1-p[neg]).sum()/3K/K
  loc:  smooth_l1(pr - encode(gt, anchor)).sum()/K
  msk:  BCE(sigmoid(coef@proto), goalmask).mean(hw).sum(k)/K
        = [ sum softplus(z) - sum_k <y_gk, z_k> ] / 16384 / K

z bulk (16 rounds x 8 slots over one manually-rotated 8-bank PSUM tile):
PE bf16 matmuls (proto chunk stationary [4,128], coefT|caggT streaming
[4,220]); exp on ACT straight from PSUM; products-of-8 of (1+e) on DVE
(bf16: pair/quad per round, oct across round pairs); Ln+accumulate in 3
groups (8/6/2 rounds) so the tail Ln is tiny.

S2 = sum_k <y_gk, z_k> = <c_agg, G>: the z matmul rhs carries 20 extra
columns c_aggT, so each z chunk also emits caggP[b, u] = <c_agg_b,
proto_u>; one fused scalar_tensor_tensor per round dots those PSUM
columns against the bf16 GT masks (host-transposed [v, t, b] layout)
with a per-partition accumulator. (tensor_tensor_reduce crashes the HW
DGE path; scalar_tensor_tensor with accum_out is the working fused op.)

Head: indices ship as f32 row-vectors [2, K]/[5, 120] (two fat DMA
descriptors instead of 128 tiny ones), transposed to per-partition
layout on the PE via scratch corners of the z PSUM tile, then cast to
i32 for the gpsimd indirect gathers (pos rows of the packed
[box|coef|ac|ahw|cls] table first — they gate the coefT transpose).

No device collective: each core emits its per-image partial scalar
(weights already fold the /8); the host sums the 8 results during
unshard.
"""
import sys

sys.path.insert(0, "/opt/trn_rl_repo")
import numpy as np
from concourse import bacc, mybir, tile
from concourse.masks import make_identity

# All ACT funcs used here (Exp, Ln) live in natural_log_exp_and_others.
# Pin the combined set so the table-load pass never alternates.
_orig_gat = bacc.get_activation_tables


def _gat_one_set(arch):
    t = _orig_gat(arch)
    keep = "natural_log_exp_and_others"
    return {k: (v if k == keep else set()) for k, v in t.items()}


bacc.get_activation_tables = _gat_one_set

N, A, K, B, P, HW = 8, 16368, 200, 20, 4, 128
HW2 = HW * HW  # 16384
KN = 3 * K  # 600
F32 = mybir.dt.float32
BF16 = mybir.dt.bfloat16
I32 = mybir.dt.int32
AF = mybir.ActivationFunctionType
ALU = mybir.AluOpType

# weights fold the final /8 mean over cores (host sums the 8 partials)
W_POS = -1.0 / (K * K * N)        # stats hold +ln(p)
W_NEG = -1.0 / (KN * K * N)       # stats hold +ln(1-p)
W_LOC = 1.0 / (K * N)
W_S1 = 1.0 / (HW2 * K * N)
W_S2 = -W_S1
INV_LN10 = float(1.0 / np.log(10.0))

ZSLOTS = 8
ZROUNDS = HW // ZSLOTS  # 16
RW = K + B  # 220: z rhs width = coefT(200) | caggT(20)


def build_kernel():
    nc = bacc.Bacc(None, target_bir_lowering=False, debug=False)

    big = nc.declare_dram_parameter("big", [A, 13], F32, isOutput=False)
    cls = nc.declare_dram_parameter("cls", [A, 1], F32, isOutput=False)
    protoNb_d = nc.declare_dram_parameter("protoNb", [P, HW2], BF16, isOutput=False)
    y2b_d = nc.declare_dram_parameter("y2b", [128, 128, B], BF16, isOutput=False)
    gtb = nc.declare_dram_parameter("gtb", [B, 4], F32, isOutput=False)
    idxf_d = nc.declare_dram_parameter("idxf", [2, K], F32, isOutput=False)
    negif_d = nc.declare_dram_parameter("negif", [5, 120], F32, isOutput=False)
    out = nc.declare_dram_parameter("out", [1, 1], F32, isOutput=True)

    with tile.TileContext(nc) as tc:
        with tc.tile_pool(name="sb", bufs=1) as sb:
            # ---------------- index loads (critical path head) ----------
            # f32 row-vector layout: 2 fat descriptors instead of 128 tiny
            # ones; transposed to per-partition layout on the PE.
            idxf = sb.tile([2, K], F32)
            nc.sync.dma_start(out=idxf[:], in_=idxf_d[:, :])
            negif = sb.tile([5, 120], F32)
            nc.sync.dma_start(out=negif[:], in_=negif_d[:, :])

            # ---------------- bulk loads (parallel queues) --------------
            protoNb = sb.tile([P, HW2], BF16)
            nc.scalar.dma_start(out=protoNb[:], in_=protoNb_d[:, :])
            # masks, bf16, layout [v, t, b]: y2b[v, t, b] = y[b, 128 t + v].
            # 4 quarter loads on distinct queues so round-0 S2 isn't gated
            # on the whole 640KB.
            y2b = sb.tile([128, 128, B], BF16)
            nc.sync.dma_start(out=y2b[:, 0:56, :], in_=y2b_d[:, 0:56, :])
            nc.sync.dma_start(out=y2b[:, 56:96, :], in_=y2b_d[:, 56:96, :])

            # ---------------- constants ---------------------------------
            ident = sb.tile([128, 128], F32)
            make_identity(nc, ident[:])
            ones = sb.tile([128, 1], F32)
            nc.vector.memset(ones[:], 1.0)
            iota_i = sb.tile([128, B], I32)
            nc.gpsimd.iota(iota_i[:], pattern=[[1, B]], base=0, channel_multiplier=0)
            iota_f = sb.tile([128, B], F32)
            nc.vector.tensor_copy(out=iota_f[:], in_=iota_i[:])
            stats = sb.tile([128, 8], F32)
            nc.vector.memset(stats[:], 0.0)
            macc = sb.tile([128, 4], F32)
            nc.vector.memset(macc[:], 0.0)
            s2acc = sb.tile([128, ZROUNDS], F32)

            # ---------------- PSUM: one manually-rotated tile ------------
            with tc.tile_pool(name="psZ", bufs=1, space="PSUM") as psZ, \
                 tc.tile_pool(name="sb2", bufs=2) as sb2:
                zp = psZ.tile([128, 2 * ZSLOTS, 256], F32)  # all 8 banks

                # index transposes: [2, K]/[5, 120] row layouts -> per-
                # partition columns via PE (scratch in zp slots 11/12,
                # consumed before round-1 matmuls overwrite them)
                idxT1 = zp[0:128, 11, 0:2]
                idxT2 = zp[0:72, 11, 4:6]
                negT = zp[0:120, 12, 0:5]
                nc.tensor.transpose(out=idxT1, in_=idxf[:, 0:128],
                                    identity=ident[0:2, 0:2])
                nc.tensor.transpose(out=idxT2, in_=idxf[:, 128:200],
                                    identity=ident[0:2, 0:2])
                nc.tensor.transpose(out=negT, in_=negif[:],
                                    identity=ident[0:5, 0:5])
                posc1 = sb.tile([128, 1], I32)
                posc2 = sb.tile([72, 1], I32)
                gtc1 = sb.tile([128, 1], I32)
                gtc2 = sb.tile([72, 1], I32)
                gidx1 = sb.tile([128, 1], F32)
                gidx2 = sb.tile([72, 1], F32)
                nc.vector.tensor_copy(out=posc1[:], in_=idxT1[:, 0:1])
                nc.vector.tensor_copy(out=gtc1[:], in_=idxT1[:, 1:2])
                nc.vector.tensor_copy(out=gidx1[:], in_=idxT1[:, 1:2])
                nc.vector.tensor_copy(out=posc2[:], in_=idxT2[:, 0:1])
                nc.vector.tensor_copy(out=gtc2[:], in_=idxT2[:, 1:2])
                nc.vector.tensor_copy(out=gidx2[:], in_=idxT2[:, 1:2])
                negc = [sb.tile([120, 1], I32, tag=f"negc{j}", name=f"negc{j}")
                        for j in range(5)]
                for j in range(5):
                    nc.vector.tensor_copy(out=negc[j][:], in_=negT[:, j:j + 1])
                bigg1 = sb.tile([128, 13], F32)
                bigg2 = sb.tile([72, 13], F32)
                nc.gpsimd.indirect_dma_start(
                    out=bigg1[:], out_offset=None, in_=big[:, :],
                    in_offset=bacc.bass.IndirectOffsetOnAxis(
                        ap=posc1[:, 0:1], axis=0))
                nc.gpsimd.indirect_dma_start(
                    out=bigg2[:], out_offset=None, in_=big[:, :],
                    in_offset=bacc.bass.IndirectOffsetOnAxis(
                        ap=posc2[:, 0:1], axis=0))
                nc.gpsimd.dma_start(out=y2b[:, 96:128, :],
                                    in_=y2b_d[:, 96:128, :])
                gtg1 = sb.tile([128, 4], F32)
                gtg2 = sb.tile([72, 4], F32)
                nc.gpsimd.indirect_dma_start(
                    out=gtg1[:], out_offset=None, in_=gtb[:, :],
                    in_offset=bacc.bass.IndirectOffsetOnAxis(
                        ap=gtc1[:, 0:1], axis=0))
                nc.gpsimd.indirect_dma_start(
                    out=gtg2[:], out_offset=None, in_=gtb[:, :],
                    in_offset=bacc.bass.IndirectOffsetOnAxis(
                        ap=gtc2[:, 0:1], axis=0))
                negp = sb.tile([120, 5], F32)
                for j in range(5):
                    nc.gpsimd.indirect_dma_start(
                        out=negp[:, j:j + 1], out_offset=None, in_=cls[:, :],
                        in_offset=bacc.bass.IndirectOffsetOnAxis(
                            ap=negc[j][:, 0:1], axis=0))

                # coefT + caggT -> rhs_all (critical path to z matmuls)
                # scratch PSUM regions live in slots 8-10, reads complete
                # before round-1 matmuls overwrite them.
                ctps1 = zp[0:P, 8, 0:128]
                ctps2 = zp[0:P, 9, 0:72]
                nc.tensor.transpose(out=ctps1, in_=bigg1[:, 4:8],
                                    identity=ident[:])
                nc.tensor.transpose(out=ctps2, in_=bigg2[:, 4:8],
                                    identity=ident[0:72, 0:72])
                rhs_all = sb.tile([P, 224], BF16)
                nc.vector.tensor_copy(out=rhs_all[:, 0:128], in_=ctps1)
                nc.vector.tensor_copy(out=rhs_all[:, 128:200], in_=ctps2)

                # c_aggT[p, b] = sum_{k: gt_k = b} coef[k, p]
                H1 = sb.tile([128, B], F32)
                H2 = sb.tile([72, B], F32)
                nc.vector.tensor_scalar(out=H1[:], in0=iota_f[:],
                                        scalar1=gidx1[:, 0:1], scalar2=None,
                                        op0=ALU.is_equal)
                nc.vector.tensor_scalar(out=H2[:], in0=iota_f[0:72, :],
                                        scalar1=gidx2[:, 0:1], scalar2=None,
                                        op0=ALU.is_equal)
                caggT = zp[0:P, 10, 0:B]
                nc.tensor.matmul(out=caggT, lhsT=bigg1[:, 4:8], rhs=H1[:],
                                 start=True, stop=False)
                nc.tensor.matmul(out=caggT, lhsT=bigg2[:, 4:8], rhs=H2[:],
                                 start=False, stop=True)
                nc.vector.tensor_copy(out=rhs_all[:, 200:220], in_=caggT)

                exp_instrs = []
                small_act = []
                small_dve = []

                def emit_small_loss():
                    # ---- classification + localization, off the bulk path.
                    # All Ln inputs batched into one staging tile -> one
                    # ACT call; elementwise stays on DVE (tracked so it can
                    # be re-anchored into round slack).
                    # staging cols: 0 p_pos1 | 1 p_pos2 | 2:4 ahw1 | 4:6 gthw1
                    #   | 6:8 ahw2 | 8:10 gthw2 ; negs separate (need accum).
                    lnin = sb.tile([128, 10], F32)
                    small_dve.append(nc.vector.memset(lnin[:], 1.0))
                    small_dve.append(nc.vector.tensor_copy(
                        out=lnin[0:128, 0:1], in_=bigg1[:, 12:13]))
                    small_dve.append(nc.vector.tensor_copy(
                        out=lnin[0:72, 1:2], in_=bigg2[:, 12:13]))
                    small_dve.append(nc.vector.tensor_copy(
                        out=lnin[0:128, 2:4], in_=bigg1[:, 10:12]))
                    small_dve.append(nc.vector.tensor_copy(
                        out=lnin[0:128, 4:6], in_=gtg1[:, 2:4]))
                    small_dve.append(nc.vector.tensor_copy(
                        out=lnin[0:72, 6:8], in_=bigg2[:, 10:12]))
                    small_dve.append(nc.vector.tensor_copy(
                        out=lnin[0:72, 8:10], in_=gtg2[:, 2:4]))
                    lng = sb.tile([128, 10], F32)
                    small_act.append(nc.scalar.activation(lng[:], lnin[:], AF.Ln))
                    small_dve.append(nc.vector.tensor_scalar(
                        out=stats[0:128, 0:1], in0=lng[0:128, 0:1],
                        scalar1=W_POS, scalar2=None, op0=ALU.mult))
                    small_dve.append(nc.vector.tensor_scalar(
                        out=stats[0:72, 1:2], in0=lng[0:72, 1:2],
                        scalar1=W_POS, scalar2=None, op0=ALU.mult))
                    # negatives: -ln(1-p) with accumulate
                    lneg = sb.tile([120, 5], F32)
                    lnegacc = sb.tile([120, 1], F32)
                    small_act.append(
                        nc.scalar.activation(lneg[:], negp[:], AF.Ln, bias=1.0,
                                             scale=-1.0, accum_out=lnegacc[:]))
                    small_dve.append(nc.vector.tensor_scalar(
                        out=stats[0:120, 2:3], in0=lnegacc[:],
                        scalar1=W_NEG, scalar2=None, op0=ALU.mult))

                    # ---- localization --------------------------------
                    # big cols: 0:4 pr, 4:8 coef, 8:10 ac, 10:12 ahw, 12 cls
                    for ci, (bigg, gtg, q, col, lo) in enumerate(
                            [(bigg1, gtg1, 128, 3, 2), (bigg2, gtg2, 72, 4, 6)]):
                        inv = sb.tile([128, 2], F32, tag=f"inv{ci}", name=f"inv{ci}")
                        small_dve.append(nc.vector.reciprocal(
                            inv[0:q, :], bigg[:, 10:12]))
                        tgt = sb.tile([128, 4], F32, tag=f"tgt{ci}", name=f"tgt{ci}")
                        small_dve.append(nc.vector.tensor_tensor(
                            out=tgt[0:q, 0:2], in0=gtg[:, 0:2],
                            in1=bigg[:, 8:10], op=ALU.subtract))
                        small_dve.append(nc.vector.tensor_tensor(
                            out=tgt[0:q, 0:2], in0=tgt[0:q, 0:2],
                            in1=inv[0:q, :], op=ALU.mult))
                        small_dve.append(nc.vector.tensor_tensor(
                            out=tgt[0:q, 2:4], in0=lng[0:q, lo + 2:lo + 4],
                            in1=lng[0:q, lo:lo + 2], op=ALU.subtract))
                        small_dve.append(nc.vector.tensor_scalar(
                            out=tgt[0:q, 2:4], in0=tgt[0:q, 2:4],
                            scalar1=INV_LN10, scalar2=None, op0=ALU.mult))
                        d = sb.tile([128, 4], F32, tag=f"d{ci}", name=f"d{ci}")
                        small_dve.append(nc.vector.tensor_tensor(
                            out=d[0:q, :], in0=bigg[:, 0:4],
                            in1=tgt[0:q, :], op=ALU.subtract))
                        nd = sb.tile([128, 4], F32, tag=f"nd{ci}", name=f"nd{ci}")
                        small_dve.append(nc.vector.tensor_scalar(
                            out=nd[0:q, :], in0=d[0:q, :],
                            scalar1=-1.0, scalar2=None, op0=ALU.mult))
                        ad = sb.tile([128, 4], F32, tag=f"ad{ci}", name=f"ad{ci}")
                        small_dve.append(nc.vector.tensor_tensor(
                            out=ad[0:q, :], in0=d[0:q, :],
                            in1=nd[0:q, :], op=ALU.max))
                        m = sb.tile([128, 4], F32, tag=f"m{ci}", name=f"m{ci}")
                        small_dve.append(nc.vector.tensor_scalar(
                            out=m[0:q, :], in0=ad[0:q, :],
                            scalar1=1.0, scalar2=None, op0=ALU.min))
                        mm = sb.tile([128, 4], F32, tag=f"mm{ci}", name=f"mm{ci}")
                        small_dve.append(nc.vector.tensor_tensor(
                            out=mm[0:q, :], in0=m[0:q, :],
                            in1=m[0:q, :], op=ALU.mult))
                        small_dve.append(nc.vector.tensor_scalar(
                            out=mm[0:q, :], in0=mm[0:q, :],
                            scalar1=0.5, scalar2=None, op0=ALU.mult))
                        small_dve.append(nc.vector.tensor_tensor(
                            out=ad[0:q, :], in0=ad[0:q, :],
                            in1=m[0:q, :], op=ALU.subtract))
                        small_dve.append(nc.vector.tensor_tensor(
                            out=mm[0:q, :], in0=mm[0:q, :],
                            in1=ad[0:q, :], op=ALU.add))
                        red = sb.tile([128, 1], F32, tag=f"red{ci}", name=f"red{ci}")
                        small_dve.append(nc.vector.tensor_reduce(
                            out=red[0:q, :], in_=mm[0:q, :],
                            axis=mybir.AxisListType.X, op=ALU.add))
                        small_dve.append(nc.vector.tensor_scalar(
                            out=stats[0:q, col:col + 1],
                            in0=red[0:q, :], scalar1=W_LOC,
                            scalar2=None, op0=ALU.mult))

                # ---------------- z rounds ---------------------------
                # products-of-8 (1+e): pair->quad per round, oct across
                # round pairs; Ln groups split 8/6/2 so the tail Ln is tiny
                LN_GROUPS = [(0, 8), (8, 14), (14, 16)]
                vbuf = None
                g_idx = 0
                ttr_trash = sb.tile([128, 2, ZSLOTS, B], F32)
                q4buf = sb.tile([128, 2, 2 * K], BF16)
                for r in range(ZROUNDS):
                    base = (r % 2) * ZSLOTS
                    for s in range(ZSLOTS):
                        t = r * ZSLOTS + s
                        nc.tensor.matmul(out=zp[:, base + s, 0:RW],
                                         lhsT=protoNb[:, t * 128:(t + 1) * 128],
                                         rhs=rhs_all[0:P, 0:RW],
                                         start=True, stop=True)
                    if r == LN_GROUPS[g_idx][0]:
                        vbuf = sb2.tile([128, 4, 2 * K], BF16, tag="vbuf",
                                        name="vbuf")
                    if r == 8:
                        emit_small_loss()
                    et = sb2.tile([128, ZSLOTS * K], BF16, tag="et", name="et")
                    ei = nc.scalar.activation(et[:], zp[:, base:base + ZSLOTS, 0:K],
                                              AF.Exp)
                    exp_instrs.append(ei)
                    # S2: dot the caggP columns with the GT masks (fused)
                    nc.vector.scalar_tensor_tensor(
                        out=ttr_trash[:, r % 2, :, :],
                        in0=zp[:, base:base + ZSLOTS, K:RW],
                        scalar=1.0,
                        in1=y2b[:, r * ZSLOTS:(r + 1) * ZSLOTS, :],
                        op0=ALU.mult, op1=ALU.mult,
                        accum_out=s2acc[:, r:r + 1])
                    nc.vector.tensor_scalar_add(et[:], et[:], 1.0)
                    t8 = sb2.tile([128, 4 * K], BF16, tag="t8", name="t8")
                    nc.vector.tensor_tensor(out=t8[:], in0=et[:, 0:4 * K],
                                            in1=et[:, 4 * K:8 * K], op=ALU.mult)
                    nc.vector.tensor_tensor(
                        out=q4buf[:, r % 2, :],
                        in0=t8[:, 0:2 * K], in1=t8[:, 2 * K:4 * K], op=ALU.mult)
                    if r % 2 == 1:
                        pr = (r - LN_GROUPS[g_idx][0]) // 2
                        nc.vector.tensor_tensor(
                            out=vbuf[:, pr, :], in0=q4buf[:, 0, :],
                            in1=q4buf[:, 1, :], op=ALU.mult)
                    if r == LN_GROUPS[g_idx][1] - 1:
                        npr = (LN_GROUPS[g_idx][1] - LN_GROUPS[g_idx][0]) // 2
                        scr = sb2.tile([128, 4, 2 * K], F32, tag="scr",
                                       name="scr")
                        nc.scalar.activation(
                            scr[:, 0:npr, :], vbuf[:, 0:npr, :], AF.Ln,
                            accum_out=macc[:, g_idx:g_idx + 1])
                        g_idx += 1

                for si in small_act:
                    tile.add_dep_helper(si.ins, exp_instrs[6].ins, sync=False,
                                        reason="small lns after z stream start")
                prev = None
                for gi, di in enumerate(small_dve):
                    anchor = exp_instrs[min(6 + gi // 5, ZROUNDS - 1)]
                    tile.add_dep_helper(di.ins, anchor.ins, sync=False,
                                        reason="spread small dve over rounds")
                    if prev is not None:
                        tile.add_dep_helper(di.ins, prev.ins, sync=False,
                                            reason="small dve chain")
                    prev = di

                # ---------------- final combine ----------------------
                s2r = sb.tile([128, 1], F32)
                nc.vector.tensor_reduce(out=s2r[:], in_=s2acc[:],
                                        axis=mybir.AxisListType.X, op=ALU.add)
                nc.vector.tensor_scalar(out=stats[:, 5:6], in0=s2r[:],
                                        scalar1=W_S2, scalar2=None, op0=ALU.mult)
                m1 = sb.tile([128, 1], F32)
                nc.vector.tensor_reduce(out=m1[:], in_=macc[:],
                                        axis=mybir.AxisListType.X, op=ALU.add)
                nc.vector.tensor_scalar(out=stats[:, 6:7], in0=m1[:], scalar1=W_S1,
                                        scalar2=None, op0=ALU.mult)
                total = sb.tile([128, 1], F32)
                nc.vector.tensor_reduce(out=total[:], in_=stats[:],
                                        axis=mybir.AxisListType.X, op=ALU.add)
                totps = zp[0:1, 0, 0:1]
                nc.tensor.matmul(out=totps, lhsT=total[:], rhs=ones[:],
                                 start=True, stop=True)
                fin = sb.tile([1, 1], F32)
                nc.vector.tensor_copy(out=fin[:], in_=totps)
                nc.sync.dma_start(out=out[:, :], in_=fin[0:1, 0:1])

    nc.finalize()
    return nc


def make_in_maps(map_class, map_box, map_coef, proto, anchor_center, anchor_hw,
                 gt_boxes, gt_masks, pos_idx, neg_idx, gt_idx):
    import ml_dtypes
    bf16 = ml_dtypes.bfloat16
    in_maps = []
    for i in range(N):
        big = np.concatenate(
            [map_box[i], map_coef[i], anchor_center, anchor_hw,
             map_class[i].reshape(A, 1)], axis=1).astype(np.float32)
        # y2b[v, t, b] = y[b, 128 t + v]
        yT = gt_masks[i].reshape(B, HW2).T            # [16384, B]
        y2b = np.ascontiguousarray(
            yT.reshape(128, 128, B).transpose(1, 0, 2)).astype(bf16)
        idxf = np.stack([pos_idx[i], gt_idx[i]], axis=0).astype(np.float32)
        in_maps.append(dict(
            big=np.ascontiguousarray(big),
            cls=np.ascontiguousarray(map_class[i].reshape(A, 1)),
            protoNb=np.ascontiguousarray(proto[i].reshape(P, HW2)).astype(bf16),
            y2b=y2b,
            gtb=np.ascontiguousarray(gt_boxes[i]),
            idxf=np.ascontiguousarray(idxf),
            negif=np.ascontiguousarray(
                neg_idx[i].reshape(5, 120).astype(np.float32)),
        ))
    return in_maps


def kernel(**inputs):
    from concourse.bass_utils import run_bass_kernel_spmd
    nc = build_kernel()
    in_maps = make_in_maps(**inputs)
    res = run_bass_kernel_spmd(nc, in_maps, core_ids=list(range(N)))
    return np.float32(sum(float(res.results[c]["out"][0, 0]) for c in range(N)))


# revision 53
# speedup vs baseline: 1.0096x; 1.0096x over previous
"""Distributed Bass kernel for nn_AllLoss: YOLACT-style loss over 8 cores.

Per-core (one image each):
  cls:  -ln(p[pos]).sum()/K/K  +  -ln(1-p[neg]).sum()/3K/K
  loc:  smooth_l1(pr - encode(gt, anchor)).sum()/K
  msk:  BCE(sigmoid(coef@proto), goalmask).mean(hw).sum(k)/K
        = [ sum softplus(z) - sum_k <y_gk, z_k> ] / 16384 / K

z bulk (16 rounds x 8 slots over one manually-rotated 8-bank PSUM tile):
PE bf16 matmuls (proto chunk stationary [4,128], coefT|caggT streaming
[4,220]); exp on ACT straight from PSUM; products-of-8 of (1+e) on DVE
(bf16: pair/quad per round, oct across round pairs); Ln+accumulate in 3
groups (8/6/2 rounds) so the tail Ln is tiny.

S2 = sum_k <y_gk, z_k> = <c_agg, G>: the z matmul rhs carries 20 extra
columns c_aggT, so each z chunk also emits caggP[b, u] = <c_agg_b,
proto_u>; one fused scalar_tensor_tensor per round dots those PSUM
columns against the bf16 GT masks (host-transposed [v, t, b] layout)
with a per-partition accumulator. (tensor_tensor_reduce crashes the HW
DGE path; scalar_tensor_tensor with accum_out is the working fused op.)

Head: indices ship as f32 row-vectors [2, K]/[5, 120] (two fat DMA
descriptors instead of 128 tiny ones), transposed to per-partition
layout on the PE via scratch corners of the z PSUM tile, then cast to
i32 for the gpsimd indirect gathers (pos rows of the packed
[box|coef|ac|ahw|cls] table first — they gate the coefT transpose).

No device collective: each core emits its per-image partial scalar
(weights already fold the /8); the host sums the 8 results during
unshard.
"""
import sys

sys.path.insert(0, "/opt/trn_rl_repo")
import numpy as np
from concourse import bacc, mybir, tile
from concourse.masks import make_identity

# All ACT funcs used here (Exp, Ln) live in natural_log_exp_and_others.
# Pin the combined set so the table-load pass never alternates.
_orig_gat = bacc.get_activation_tables


def _gat_one_set(arch):
    t = _orig_gat(arch)
    keep = "natural_log_exp_and_others"
    return {k: (v if k == keep else set()) for k, v in t.items()}


bacc.get_activation_tables = _gat_one_set

N, A, K, B, P, HW = 8, 16368, 200, 20, 4, 128
HW2 = HW * HW  # 16384
KN = 3 * K  # 600
F32 = mybir.dt.float32
BF16 = mybir.dt.bfloat16
I32 = mybir.dt.int32
AF = mybir.ActivationFunctionType
ALU = mybir.AluOpType

# weights fold the final /8 mean over cores (host sums the 8 partials)
W_POS = -1.0 / (K * K * N)        # stats hold +ln(p)
W_NEG = -1.0 / (KN * K * N)       # stats hold +ln(1-p)
W_LOC = 1.0 / (K * N)
W_S1 = 1.0 / (HW2 * K * N)
W_S2 = -W_S1
INV_LN10 = float(1.0 / np.log(10.0))

ZSLOTS = 8
ZROUNDS = HW // ZSLOTS  # 16
RW = K + B  # 220: z rhs width = coefT(200) | caggT(20)


def build_kernel():
    nc = bacc.Bacc(None, target_bir_lowering=False, debug=False)

    big = nc.declare_dram_parameter("big", [A, 13], F32, isOutput=False)
    cls = nc.declare_dram_parameter("cls", [A, 1], F32, isOutput=False)
    protoNb_d = nc.declare_dram_parameter("protoNb", [P, HW2], BF16, isOutput=False)
    y2b_d = nc.declare_dram_parameter("y2b", [128, 128, B], BF16, isOutput=False)
    gtb = nc.declare_dram_parameter("gtb", [B, 4], F32, isOutput=False)
    idxf_d = nc.declare_dram_parameter("idxf", [2, K], F32, isOutput=False)
    negif_d = nc.declare_dram_parameter("negif", [5, 120], F32, isOutput=False)
    out = nc.declare_dram_parameter("out", [1, 1], F32, isOutput=True)

    with tile.TileContext(nc) as tc:
        with tc.tile_pool(name="sb", bufs=1) as sb:
            # ---------------- index loads (critical path head) ----------
            # f32 row-vector layout: 2 fat descriptors instead of 128 tiny
            # ones; transposed to per-partition layout on the PE.
            idxf = sb.tile([2, K], F32)
            nc.sync.dma_start(out=idxf[:], in_=idxf_d[:, :])
            negif = sb.tile([5, 120], F32)
            nc.sync.dma_start(out=negif[:], in_=negif_d[:, :])

            # ---------------- bulk loads (parallel queues) --------------
            protoNb = sb.tile([P, HW2], BF16)
            nc.scalar.dma_start(out=protoNb[:], in_=protoNb_d[:, :])
            # masks, bf16, layout [v, t, b]: y2b[v, t, b] = y[b, 128 t + v].
            # 4 quarter loads on distinct queues so round-0 S2 isn't gated
            # on the whole 640KB.
            y2b = sb.tile([128, 128, B], BF16)
            nc.sync.dma_start(out=y2b[:, 0:56, :], in_=y2b_d[:, 0:56, :])
            nc.sync.dma_start(out=y2b[:, 56:96, :], in_=y2b_d[:, 56:96, :])

            # ---------------- constants ---------------------------------
            ident = sb.tile([128, 128], F32)
            make_identity(nc, ident[:])
            ones = sb.tile([128, 1], F32)
            nc.vector.memset(ones[:], 1.0)
            iota_i = sb.tile([128, B], I32)
            nc.gpsimd.iota(iota_i[:], pattern=[[1, B]], base=0, channel_multiplier=0)
            iota_f = sb.tile([128, B], F32)
            nc.vector.tensor_copy(out=iota_f[:], in_=iota_i[:])
            stats = sb.tile([128, 8], F32)
            nc.vector.memset(stats[:], 0.0)
            macc = sb.tile([128, 4], F32)
            nc.vector.memset(macc[:], 0.0)
            s2acc = sb.tile([128, ZROUNDS], F32)

            # ---------------- PSUM: one manually-rotated tile ------------
            with tc.tile_pool(name="psZ", bufs=1, space="PSUM") as psZ, \
                 tc.tile_pool(name="sb2", bufs=2) as sb2:
                zp = psZ.tile([128, 2 * ZSLOTS, 256], F32)  # all 8 banks

                # index transposes: [2, K]/[5, 120] row layouts -> per-
                # partition columns via PE (scratch in zp slots 11/12,
                # consumed before round-1 matmuls overwrite them)
                idxT1 = zp[0:128, 11, 0:2]
                idxT2 = zp[0:72, 11, 4:6]
                negT = zp[0:120, 12, 0:5]
                nc.tensor.transpose(out=idxT1, in_=idxf[:, 0:128],
                                    identity=ident[0:2, 0:2])
                nc.tensor.transpose(out=idxT2, in_=idxf[:, 128:200],
                                    identity=ident[0:2, 0:2])
                nc.tensor.transpose(out=negT, in_=negif[:],
                                    identity=ident[0:5, 0:5])
                # HAM warm-up: ~3us of bf16 matmuls from the early proto DMA
                # (weights ready ~10.2us), ending before the gather data
                # lands (~13.5us); output bytes 896..1016 of slot 13 are
                # disjoint from all real writes (z matmuls use 0..880).
                for w in range(18):
                    nc.tensor.matmul(out=zp[:, 13, 224:254],
                                     lhsT=protoNb[0:P, 0:128],
                                     rhs=protoNb[0:P, 0:30],
                                     start=True, stop=True)
                posc1 = sb.tile([128, 1], I32)
                posc2 = sb.tile([72, 1], I32)
                gtc1 = sb.tile([128, 1], I32)
                gtc2 = sb.tile([72, 1], I32)
                gidx1 = sb.tile([128, 1], F32)
                gidx2 = sb.tile([72, 1], F32)
                nc.vector.tensor_copy(out=posc1[:], in_=idxT1[:, 0:1])
                nc.vector.tensor_copy(out=gtc1[:], in_=idxT1[:, 1:2])
                nc.vector.tensor_copy(out=gidx1[:], in_=idxT1[:, 1:2])
                nc.vector.tensor_copy(out=posc2[:], in_=idxT2[:, 0:1])
                nc.vector.tensor_copy(out=gtc2[:], in_=idxT2[:, 1:2])
                nc.vector.tensor_copy(out=gidx2[:], in_=idxT2[:, 1:2])
                negc = [sb.tile([120, 1], I32, tag=f"negc{j}", name=f"negc{j}")
                        for j in range(5)]
                for j in range(5):
                    nc.vector.tensor_copy(out=negc[j][:], in_=negT[:, j:j + 1])
                bigg1 = sb.tile([128, 13], F32)
                bigg2 = sb.tile([72, 13], F32)
                nc.gpsimd.indirect_dma_start(
                    out=bigg1[:], out_offset=None, in_=big[:, :],
                    in_offset=bacc.bass.IndirectOffsetOnAxis(
                        ap=posc1[:, 0:1], axis=0))
                nc.gpsimd.indirect_dma_start(
                    out=bigg2[:], out_offset=None, in_=big[:, :],
                    in_offset=bacc.bass.IndirectOffsetOnAxis(
                        ap=posc2[:, 0:1], axis=0))
                nc.gpsimd.dma_start(out=y2b[:, 96:128, :],
                                    in_=y2b_d[:, 96:128, :])
                gtg1 = sb.tile([128, 4], F32)
                gtg2 = sb.tile([72, 4], F32)
                nc.gpsimd.indirect_dma_start(
                    out=gtg1[:], out_offset=None, in_=gtb[:, :],
                    in_offset=bacc.bass.IndirectOffsetOnAxis(
                        ap=gtc1[:, 0:1], axis=0))
                nc.gpsimd.indirect_dma_start(
                    out=gtg2[:], out_offset=None, in_=gtb[:, :],
                    in_offset=bacc.bass.IndirectOffsetOnAxis(
                        ap=gtc2[:, 0:1], axis=0))
                negp = sb.tile([120, 5], F32)
                for j in range(5):
                    nc.gpsimd.indirect_dma_start(
                        out=negp[:, j:j + 1], out_offset=None, in_=cls[:, :],
                        in_offset=bacc.bass.IndirectOffsetOnAxis(
                            ap=negc[j][:, 0:1], axis=0))

                # coefT + caggT -> rhs_all (critical path to z matmuls)
                # scratch PSUM regions live in slots 8-10, reads complete
                # before round-1 matmuls overwrite them.
                ctps1 = zp[0:P, 8, 0:128]
                ctps2 = zp[0:P, 9, 0:72]
                nc.tensor.transpose(out=ctps1, in_=bigg1[:, 4:8],
                                    identity=ident[:])
                nc.tensor.transpose(out=ctps2, in_=bigg2[:, 4:8],
                                    identity=ident[0:72, 0:72])
                rhs_all = sb.tile([P, 224], BF16)
                nc.vector.tensor_copy(out=rhs_all[:, 0:128], in_=ctps1)
                nc.vector.tensor_copy(out=rhs_all[:, 128:200], in_=ctps2)

                # c_aggT[p, b] = sum_{k: gt_k = b} coef[k, p]
                H1 = sb.tile([128, B], F32)
                H2 = sb.tile([72, B], F32)
                nc.vector.tensor_scalar(out=H1[:], in0=iota_f[:],
                                        scalar1=gidx1[:, 0:1], scalar2=None,
                                        op0=ALU.is_equal)
                nc.vector.tensor_scalar(out=H2[:], in0=iota_f[0:72, :],
                                        scalar1=gidx2[:, 0:1], scalar2=None,
                                        op0=ALU.is_equal)
                caggT = zp[0:P, 10, 0:B]
                nc.tensor.matmul(out=caggT, lhsT=bigg1[:, 4:8], rhs=H1[:],
                                 start=True, stop=False)
                nc.tensor.matmul(out=caggT, lhsT=bigg2[:, 4:8], rhs=H2[:],
                                 start=False, stop=True)
                nc.vector.tensor_copy(out=rhs_all[:, 200:220], in_=caggT)

                exp_instrs = []
                small_act = []
                small_dve = []

                def emit_small_loss():
                    # ---- classification + localization, off the bulk path.
                    # All Ln inputs batched into one staging tile -> one
                    # ACT call; elementwise stays on DVE (tracked so it can
                    # be re-anchored into round slack).
                    # staging cols: 0 p_pos1 | 1 p_pos2 | 2:4 ahw1 | 4:6 gthw1
                    #   | 6:8 ahw2 | 8:10 gthw2 ; negs separate (need accum).
                    lnin = sb.tile([128, 10], F32)
                    small_dve.append(nc.vector.memset(lnin[:], 1.0))
                    small_dve.append(nc.vector.tensor_copy(
                        out=lnin[0:128, 0:1], in_=bigg1[:, 12:13]))
                    small_dve.append(nc.vector.tensor_copy(
                        out=lnin[0:72, 1:2], in_=bigg2[:, 12:13]))
                    small_dve.append(nc.vector.tensor_copy(
                        out=lnin[0:128, 2:4], in_=bigg1[:, 10:12]))
                    small_dve.append(nc.vector.tensor_copy(
                        out=lnin[0:128, 4:6], in_=gtg1[:, 2:4]))
                    small_dve.append(nc.vector.tensor_copy(
                        out=lnin[0:72, 6:8], in_=bigg2[:, 10:12]))
                    small_dve.append(nc.vector.tensor_copy(
                        out=lnin[0:72, 8:10], in_=gtg2[:, 2:4]))
                    lng = sb.tile([128, 10], F32)
                    small_act.append(nc.scalar.activation(lng[:], lnin[:], AF.Ln))
                    small_dve.append(nc.vector.tensor_scalar(
                        out=stats[0:128, 0:1], in0=lng[0:128, 0:1],
                        scalar1=W_POS, scalar2=None, op0=ALU.mult))
                    small_dve.append(nc.vector.tensor_scalar(
                        out=stats[0:72, 1:2], in0=lng[0:72, 1:2],
                        scalar1=W_POS, scalar2=None, op0=ALU.mult))
                    # negatives: -ln(1-p) with accumulate
                    lneg = sb.tile([120, 5], F32)
                    lnegacc = sb.tile([120, 1], F32)
                    small_act.append(
                        nc.scalar.activation(lneg[:], negp[:], AF.Ln, bias=1.0,
                                             scale=-1.0, accum_out=lnegacc[:]))
                    small_dve.append(nc.vector.tensor_scalar(
                        out=stats[0:120, 2:3], in0=lnegacc[:],
                        scalar1=W_NEG, scalar2=None, op0=ALU.mult))

                    # ---- localization --------------------------------
                    # big cols: 0:4 pr, 4:8 coef, 8:10 ac, 10:12 ahw, 12 cls
                    for ci, (bigg, gtg, q, col, lo) in enumerate(
                            [(bigg1, gtg1, 128, 3, 2), (bigg2, gtg2, 72, 4, 6)]):
                        inv = sb.tile([128, 2], F32, tag=f"inv{ci}", name=f"inv{ci}")
                        small_dve.append(nc.vector.reciprocal(
                            inv[0:q, :], bigg[:, 10:12]))
                        tgt = sb.tile([128, 4], F32, tag=f"tgt{ci}", name=f"tgt{ci}")
                        small_dve.append(nc.vector.tensor_tensor(
                            out=tgt[0:q, 0:2], in0=gtg[:, 0:2],
                            in1=bigg[:, 8:10], op=ALU.subtract))
                        small_dve.append(nc.vector.tensor_tensor(
                            out=tgt[0:q, 0:2], in0=tgt[0:q, 0:2],
                            in1=inv[0:q, :], op=ALU.mult))
                        small_dve.append(nc.vector.tensor_tensor(
                            out=tgt[0:q, 2:4], in0=lng[0:q, lo + 2:lo + 4],
                            in1=lng[0:q, lo:lo + 2], op=ALU.subtract))
                        small_dve.append(nc.vector.tensor_scalar(
                            out=tgt[0:q, 2:4], in0=tgt[0:q, 2:4],
                            scalar1=INV_LN10, scalar2=None, op0=ALU.mult))
                        d = sb.tile([128, 4], F32, tag=f"d{ci}", name=f"d{ci}")
                        small_dve.append(nc.vector.tensor_tensor(
                            out=d[0:q, :], in0=bigg[:, 0:4],
                            in1=tgt[0:q, :], op=ALU.subtract))
                        nd = sb.tile([128, 4], F32, tag=f"nd{ci}", name=f"nd{ci}")
                        small_dve.append(nc.vector.tensor_scalar(
                            out=nd[0:q, :], in0=d[0:q, :],
                            scalar1=-1.0, scalar2=None, op0=ALU.mult))
                        ad = sb.tile([128, 4], F32, tag=f"ad{ci}", name=f"ad{ci}")
                        small_dve.append(nc.vector.tensor_tensor(
                            out=ad[0:q, :], in0=d[0:q, :],
                            in1=nd[0:q, :], op=ALU.max))
                        m = sb.tile([128, 4], F32, tag=f"m{ci}", name=f"m{ci}")
                        small_dve.append(nc.vector.tensor_scalar(
                            out=m[0:q, :], in0=ad[0:q, :],
                            scalar1=1.0, scalar2=None, op0=ALU.min))
                        mm = sb.tile([128, 4], F32, tag=f"mm{ci}", name=f"mm{ci}")
                        small_dve.append(nc.vector.tensor_tensor(
                            out=mm[0:q, :], in0=m[0:q, :],
                            in1=m[0:q, :], op=ALU.mult))
                        small_dve.append(nc.vector.tensor_scalar(
                            out=mm[0:q, :], in0=mm[0:q, :],
                            scalar1=0.5, scalar2=None, op0=ALU.mult))
                        small_dve.append(nc.vector.tensor_tensor(
                            out=ad[0:q, :], in0=ad[0:q, :],
                            in1=m[0:q, :], op=ALU.subtract))
                        small_dve.append(nc.vector.tensor_tensor(
                            out=mm[0:q, :], in0=mm[0:q, :],
                            in1=ad[0:q, :], op=ALU.add))
                        red = sb.tile([128, 1], F32, tag=f"red{ci}", name=f"red{ci}")
                        small_dve.append(nc.vector.tensor_reduce(
                            out=red[0:q, :], in_=mm[0:q, :],
                            axis=mybir.AxisListType.X, op=ALU.add))
                        small_dve.append(nc.vector.tensor_scalar(
                            out=stats[0:q, col:col + 1],
                            in0=red[0:q, :], scalar1=W_LOC,
                            scalar2=None, op0=ALU.mult))

                # ---------------- z rounds ---------------------------
                # products-of-8 (1+e): pair->quad per round, oct across
                # round pairs; Ln groups split 8/6/2 so the tail Ln is tiny
                LN_GROUPS = [(0, 8), (8, 14), (14, 16)]
                vbuf = None
                g_idx = 0
                ttr_trash = sb.tile([128, 2, ZSLOTS, B], F32)
                q4buf = sb.tile([128, 2, 2 * K], BF16)
                for r in range(ZROUNDS):
                    base = (r % 2) * ZSLOTS
                    for s in range(ZSLOTS):
                        t = r * ZSLOTS + s
                        nc.tensor.matmul(out=zp[:, base + s, 0:RW],
                                         lhsT=protoNb[:, t * 128:(t + 1) * 128],
                                         rhs=rhs_all[0:P, 0:RW],
                                         start=True, stop=True)
                    if r == LN_GROUPS[g_idx][0]:
                        vbuf = sb2.tile([128, 4, 2 * K], BF16, tag="vbuf",
                                        name="vbuf")
                    if r == 8:
                        emit_small_loss()
                    et = sb2.tile([128, ZSLOTS * K], BF16, tag="et", name="et")
                    ei = nc.scalar.activation(et[:], zp[:, base:base + ZSLOTS, 0:K],
                                              AF.Exp)
                    exp_instrs.append(ei)
                    # S2: dot the caggP columns with the GT masks (fused)
                    nc.vector.scalar_tensor_tensor(
                        out=ttr_trash[:, r % 2, :, :],
                        in0=zp[:, base:base + ZSLOTS, K:RW],
                        scalar=1.0,
                        in1=y2b[:, r * ZSLOTS:(r + 1) * ZSLOTS, :],
                        op0=ALU.mult, op1=ALU.mult,
                        accum_out=s2acc[:, r:r + 1])
                    nc.vector.tensor_scalar_add(et[:], et[:], 1.0)
                    t8 = sb2.tile([128, 4 * K], BF16, tag="t8", name="t8")
                    nc.vector.tensor_tensor(out=t8[:], in0=et[:, 0:4 * K],
                                            in1=et[:, 4 * K:8 * K], op=ALU.mult)
                    nc.vector.tensor_tensor(
                        out=q4buf[:, r % 2, :],
                        in0=t8[:, 0:2 * K], in1=t8[:, 2 * K:4 * K], op=ALU.mult)
                    if r % 2 == 1:
                        pr = (r - LN_GROUPS[g_idx][0]) // 2
                        nc.vector.tensor_tensor(
                            out=vbuf[:, pr, :], in0=q4buf[:, 0, :],
                            in1=q4buf[:, 1, :], op=ALU.mult)
                    if r == LN_GROUPS[g_idx][1] - 1:
                        npr = (LN_GROUPS[g_idx][1] - LN_GROUPS[g_idx][0]) // 2
                        scr = sb2.tile([128, 4, 2 * K], F32, tag="scr",
                                       name="scr")
                        nc.scalar.activation(
                            scr[:, 0:npr, :], vbuf[:, 0:npr, :], AF.Ln,
                            accum_out=macc[:, g_idx:g_idx + 1])
                        g_idx += 1

                for si in small_act:
                    tile.add_dep_helper(si.ins, exp_instrs[6].ins, sync=False,
                                        reason="small lns after z stream start")
                prev = None
                for gi, di in enumerate(small_dve):
                    anchor = exp_instrs[min(6 + gi // 5, ZROUNDS - 1)]
                    tile.add_dep_helper(di.ins, anchor.ins, sync=False,
                                        reason="spread small dve over rounds")
                    if prev is not None:
                        tile.add_dep_helper(di.ins, prev.ins, sync=False,
                                            reason="small dve chain")
                    prev = di

                # ---------------- final combine ----------------------
                s2r = sb.tile([128, 1], F32)
                nc.vector.tensor_reduce(out=s2r[:], in_=s2acc[:],
                                        axis=mybir.AxisListType.X, op=ALU.add)
                nc.vector.tensor_scalar(out=stats[:, 5:6], in0=s2r[:],
                                        scalar1=W_S2, scalar2=None, op0=ALU.mult)
                m1 = sb.tile([128, 1], F32)
                nc.vector.tensor_reduce(out=m1[:], in_=macc[:],
                                        axis=mybir.AxisListType.X, op=ALU.add)
                nc.vector.tensor_scalar(out=stats[:, 6:7], in0=m1[:], scalar1=W_S1,
                                        scalar2=None, op0=ALU.mult)
                total = sb.tile([128, 1], F32)
                nc.vector.tensor_reduce(out=total[:], in_=stats[:],
                                        axis=mybir.AxisListType.X, op=ALU.add)
                totps = zp[0:1, 0, 0:1]
                nc.tensor.matmul(out=totps, lhsT=total[:], rhs=ones[:],
                                 start=True, stop=True)
                fin = sb.tile([1, 1], F32)
                nc.vector.tensor_copy(out=fin[:], in_=totps)
                nc.sync.dma_start(out=out[:, :], in_=fin[0:1, 0:1])

    nc.finalize()
    return nc


def make_in_maps(map_class, map_box, map_coef, proto, anchor_center, anchor_hw,
                 gt_boxes, gt_masks, pos_idx, neg_idx, gt_idx):
    import ml_dtypes
    bf16 = ml_dtypes.bfloat16
    in_maps = []
    for i in range(N):
        big = np.concatenate(
            [map_box[i], map_coef[i], anchor_center, anchor_hw,
             map_class[i].reshape(A, 1)], axis=1).astype(np.float32)
        # y2b[v, t, b] = y[b, 128 t + v]
        yT = gt_masks[i].reshape(B, HW2).T            # [16384, B]
        y2b = np.ascontiguousarray(
            yT.reshape(128, 128, B).transpose(1, 0, 2)).astype(bf16)
        idxf = np.stack([pos_idx[i], gt_idx[i]], axis=0).astype(np.float32)
        in_maps.append(dict(
            big=np.ascontiguousarray(big),
            cls=np.ascontiguousarray(map_class[i].reshape(A, 1)),
            protoNb=np.ascontiguousarray(proto[i].reshape(P, HW2)).astype(bf16),
            y2b=y2b,
            gtb=np.ascontiguousarray(gt_boxes[i]),
            idxf=np.ascontiguousarray(idxf),
            negif=np.ascontiguousarray(
                neg_idx[i].reshape(5, 120).astype(np.float32)),
        ))
    return in_maps


def kernel(**inputs):
    from concourse.bass_utils import run_bass_kernel_spmd
    nc = build_kernel()
    in_maps = make_in_maps(**inputs)
    res = run_bass_kernel_spmd(nc, in_maps, core_ids=list(range(N)))
    return np.float32(sum(float(res.results[c]["out"][0, 0]) for c in range(N)))
